# revision 1
# baseline (speedup 1.0000x reference)
"""Trainium2 Bass kernel for nn_CCE_Head (B=8, C=512, N=19, H=W=128).

Data-parallel over batch: one sample per NeuronCore (8 cores).

v1 design:
 - x [C, HW] bf16 resident in SBUF (conv3 stationaries + fin-conv moving).
 - xT uploaded from host as fp8 in SBUF layout [128, NCH, C], streamed in
   double-buffered group tiles (ocr moving operand). No PE transpose.
 - E = exp(probs - 2) stored fp8; ocr runs DoubleRow fp8 (chunk pairs,
   K=256) into 4 psum column strips.
 - LayerNorm rsqrt via DVE bit-trick + Newton (no Ln -> single ACT table
   set for the whole kernel).
 - fin conv: 1024-col moving matmuls, bias added during psum evacuation
   on alternating scalar/vector engines.
"""

import numpy as np

import concourse.bacc as bacc
import concourse.bass as bass
import concourse.tile as tile
from concourse import mybir
from concourse import bass_utils

F32 = mybir.dt.float32
BF16 = mybir.dt.bfloat16
F8 = mybir.dt.float8e4
U32 = mybir.dt.uint32
AF = mybir.ActivationFunctionType
ALU = mybir.AluOpType
DR = mybir.MatmulPerfMode.DoubleRow

B, C, N, H, W = 8, 512, 19, 128, 128
HW = H * W                      # 16384
CB = C // 128                   # 4 c-tiles
NCH = HW // 128                 # 128 pixel chunks of 128
GRP = 16                        # chunks per group
NGRP = NCH // GRP               # 8 groups
ASLOT = 64                      # psumA per-chunk slot (f32)
FSLOT = 20                      # per-field column slot (19 used + 1 pad)
FS = 32                         # E_sb per-chunk slot (fp8, 16B-aligned pairs)
ESHIFT = -2.0                   # E = exp(probs + ESHIFT); cancels in softmax
LN_EPS = 1e-5
PCW = 512                       # fin conv moving width (psum bank limit)
NPC = HW // PCW                 # 32
DMAC = 4                        # fin chunks per output DMA


def build_module(repeat=1, feats=("ident", "chain", "ocr", "head", "fin",
                                  "evacb", "xload")):
    feats = frozenset(feats)
    nc = bacc.Bacc("TRN2", target_bir_lowering=False, debug=False,
                   enable_asserts=False)

    x_d = nc.dram_tensor("x", [C, HW], BF16, kind="ExternalInput")
    xt8_d = nc.dram_tensor("xt8", [128, NCH * C], F8, kind="ExternalInput")
    wt_d = nc.dram_tensor("wt60", [C, 3 * FSLOT], F32, kind="ExternalInput")
    bias_d = nc.dram_tensor("bias60", [3 * FSLOT], F32, kind="ExternalInput")
    maskw_d = nc.dram_tensor("maskw", [C], F32, kind="ExternalInput")
    cm1t_d = nc.dram_tensor("cm1T", [C, C], F32, kind="ExternalInput")
    cm1b_d = nc.dram_tensor("cm1b", [C], F32, kind="ExternalInput")
    lng_d = nc.dram_tensor("lng", [C], F32, kind="ExternalInput")
    lnb_d = nc.dram_tensor("lnb", [C], F32, kind="ExternalInput")
    cm2t_d = nc.dram_tensor("cm2T", [C, C], F32, kind="ExternalInput")
    cm2b_d = nc.dram_tensor("cm2b", [C], F32, kind="ExternalInput")
    finwt_d = nc.dram_tensor("finWT", [C, N], F32, kind="ExternalInput")
    finb_d = nc.dram_tensor("finb", [32], F32, kind="ExternalInput")
    out_d = nc.dram_tensor("out", [N, HW], BF16, kind="ExternalOutput")

    def bcast_ap(handle, reps, inner):
        a = handle.ap()
        return bass.AP(tensor=a.tensor, offset=a.offset,
                       ap=[[0, 128], [0, reps], [1, inner]])

    with tile.TileContext(nc) as tc:
        import contextlib
        with contextlib.ExitStack() as ctx:
            const = ctx.enter_context(tc.tile_pool(name="const", bufs=1))
            psum_keep = ctx.enter_context(
                tc.tile_pool(name="psum_keep", bufs=1, space="PSUM"))

            # ---------------- constants / weights ----------------
            wt_sb = const.tile([128, CB, 3 * FSLOT], BF16)
            nc.gpsimd.dma_start(
                out=wt_sb, in_=wt_d.ap().rearrange("(cb p) f -> p cb f", cb=CB))

            bias_grp = const.tile([128, GRP, 3 * FSLOT], F32)
            nc.gpsimd.dma_start(out=bias_grp, in_=bcast_ap(bias_d, GRP, 3 * FSLOT))

            maskw_sb = const.tile([N, C], F32)
            a = maskw_d.ap()
            nc.gpsimd.dma_start(
                out=maskw_sb,
                in_=bass.AP(tensor=a.tensor, offset=a.offset, ap=[[0, N], [1, C]]))

            cm1t_sb = const.tile([128, CB, C], BF16)
            nc.gpsimd.dma_start(
                out=cm1t_sb, in_=cm1t_d.ap().rearrange("(cb p) m -> p cb m", cb=CB))
            cm2t_sb = const.tile([128, CB, C], BF16)
            nc.gpsimd.dma_start(
                out=cm2t_sb, in_=cm2t_d.ap().rearrange("(mb p) c -> p mb c", mb=CB))
            finwt_sb = const.tile([128, CB, N], F32)
            nc.sync.dma_start(
                out=finwt_sb, in_=finwt_d.ap().rearrange("(cb p) n -> p cb n", cb=CB))

            def col128(handle):
                t = const.tile([128, CB], F32, name=f"{handle.name}_sb")
                a = handle.ap()
                nc.gpsimd.dma_start(
                    out=t, in_=bass.AP(tensor=a.tensor, offset=a.offset,
                                       ap=[[1, 128], [128, CB]]))
                return t

            cm1b_sb = col128(cm1b_d)
            lng_sb = col128(lng_d)
            lnb_sb = col128(lnb_d)
            cm2b_sb = col128(cm2b_d)

            finb_sb = const.tile([N, 1], F32)
            a = finb_d.ap()
            nc.gpsimd.dma_start(
                out=finb_sb,
                in_=bass.AP(tensor=a.tensor, offset=a.offset, ap=[[1, N], [0, 1]]))
            finb_bc = const.tile([128, 1], F32)
            nc.gpsimd.dma_start(
                out=finb_bc,
                in_=bass.AP(tensor=a.tensor, offset=a.offset,
                            ap=[[0, 4], [1, 32], [0, 1]]))

            ones_bf = const.tile([128, 1], BF16)
            nc.vector.memset(ones_bf, 1.0)
            ones_f8 = const.tile([128, 1], F8)
            nc.vector.memset(ones_f8, 1.0)
            ones_col = const.tile([128, 1], F32)
            nc.vector.memset(ones_col, 1.0)
            ones_row = const.tile([1, 128], F32)
            nc.vector.memset(ones_row, 1.0)
            one1 = const.tile([1, 1], F32)
            nc.vector.memset(one1, 1.0)
            ones19 = const.tile([N, 1], F32)
            nc.vector.memset(ones19, 1.0)
            eshift_bias = const.tile([128, 1], F32)
            nc.vector.memset(eshift_bias, 0.5 + ESHIFT)

            # rsqrt bit-trick constants
            magic_u = const.tile([1, 1], U32)
            nc.vector.memset(magic_u, 0x5f3759df)
            one_u = const.tile([1, 1], U32)
            nc.vector.memset(one_u, 1)
            c15 = const.tile([1, 1], F32)
            nc.vector.memset(c15, 1.5)

            # pin exp_and_others as the only ACT table set
            dummy = const.tile([1, 1], F32)
            nc.vector.memset(dummy, 0.0)
            nc.scalar.activation(out=dummy, in_=dummy, func=AF.Exp)

            # per-group E tiles (fp8); cols [20:32) stay zero forever
            e_g = []
            for g in range(NGRP):
                t = const.tile([128, GRP, FS], F8, name=f"e{g}")
                nc.vector.memset(t, 0.0)
                e_g.append(t)

            # ---------------- x load (one tile per stripe/group) ----------------
            xr = x_d.ap().rearrange("(cb p) q -> p cb q", cb=CB)
            NSTRIPE = NGRP
            sw = HW // NSTRIPE
            x_str = [const.tile([128, CB, sw], BF16, name=f"xs{s}")
                     for s in range(NSTRIPE)]
            if "xload" in feats and "xin" not in feats:
                for s in range(NSTRIPE):
                    nc.sync.dma_start(out=x_str[s],
                                      in_=xr[:, :, s * sw:(s + 1) * sw])

            # persistent accumulators
            psum_ocrp = psum_keep.tile([32, C], F32)
            nc.vector.memset(psum_ocrp, 0.0)
            psum_sums = psum_keep.tile([1, GRP * FS], F32)
            if "ocr" not in feats:
                nc.vector.memset(psum_sums, 1.0)

            out_ring = ctx.enter_context(tc.tile_pool(name="out_ring", bufs=2))
            head_sb = ctx.enter_context(tc.tile_pool(name="head_sb", bufs=1))

            def main_body():
                if "xin" in feats:
                    for s in range(NSTRIPE):
                        nc.sync.dma_start(out=x_str[s],
                                          in_=xr[:, :, s * sw:(s + 1) * sw])
                with contextlib.ExitStack() as mctx:
                    grp_pool = mctx.enter_context(
                        tc.tile_pool(name="grp", bufs=3))
                    xtg_pool = mctx.enter_context(
                        tc.tile_pool(name="xtg", bufs=4))
                    psA_pool = mctx.enter_context(
                        tc.tile_pool(name="psA", bufs=3, space="PSUM"))

                    OLAG = 2       # ocr trails conv3/chain by this many groups
                    xtgs = {}

                    def emit_ocr(g):
                        # softmax denominator partial sums (reads quantized E)
                        nc.tensor.matmul(psum_sums, ones_f8, e_g[g],
                                         start=(g == 0), stop=(g == NGRP - 1),
                                         skip_group_check=True)
                        # ocr accumulation: DoubleRow over chunk pairs (K=256)
                        for t in range(GRP // 2):
                            pair = g * (GRP // 2) + t
                            nc.tensor.matmul(
                                psum_ocrp,
                                e_g[g][:, 2 * t:2 * t + 2, :],
                                xtgs[g][:, 2 * t:2 * t + 2, :],
                                start=(pair == 0),
                                stop=(pair == NCH // 2 - 1),
                                perf_mode=DR,
                                skip_group_check=True)

                    for g in range(NGRP):
                        gsl = slice(g * GRP, (g + 1) * GRP)
                        # stream this group's xT slab (fp8, host layout)
                        xtgs[g] = xtg_pool.tile([128, GRP, C], F8, name="xtg")
                        if "ocr" in feats:
                            nc.gpsimd.dma_start(
                                out=xtgs[g],
                                in_=xt8_d.ap()[:, g * GRP * C:(g + 1) * GRP * C]
                                .rearrange("p (j c) -> p j c", j=GRP))

                        psA = psA_pool.tile([128, GRP, ASLOT], F32, name="psA")
                        grp = grp_pool.tile([128, GRP, 4 * FSLOT], BF16, name="grp")
                        for j in range(GRP):
                            for cb in range(CB):
                                nc.tensor.matmul(psA[:, j, 0:3 * FSLOT],
                                                 x_str[g][:, cb, j * 128:(j + 1) * 128],
                                                 wt_sb[:, cb, :],
                                                 start=(cb == 0), stop=(cb == CB - 1))

                        # pipelined ocr for an earlier, chain-complete group
                        if "chain" in feats and "ocr" in feats and g >= OLAG:
                            emit_ocr(g - OLAG)

                        if "chain" not in feats:
                            nc.vector.tensor_add(grp[:, :, 0:3 * FSLOT],
                                                 psA[:, :, 0:3 * FSLOT], bias_grp)
                            continue
                        # sigmoid chain via tanh, two independent half-slabs
                        # interleaved so scalar/vector pipeline each other
                        HG = GRP // 2
                        halves = []
                        for h in range(2):
                            hsl = slice(h * HG, (h + 1) * HG)
                            halves.append(dict(
                                m_=grp[:, hsl, 0:FSLOT],
                                d_=grp[:, hsl, FSLOT:2 * FSLOT],
                                bd_=grp[:, hsl, 2 * FSLOT:3 * FSLOT],
                                tmp=grp[:, hsl, 3 * FSLOT:4 * FSLOT],
                                ps=psA[:, hsl, 0:3 * FSLOT],
                                full=grp[:, hsl, 0:3 * FSLOT],
                                dbd=grp[:, hsl, FSLOT:3 * FSLOT],
                                bias=bias_grp[:, hsl, :],
                                e=e_g[g][:, hsl, 0:FSLOT]))

                        def step(fn):
                            for hv in halves:
                                fn(hv)

                        step(lambda v: nc.vector.tensor_add(
                            v["m_"], v["ps"][:, :, 0:FSLOT],
                            v["bias"][:, :, 0:FSLOT]))
                        step(lambda v: nc.scalar.activation(
                            out=v["tmp"], in_=v["m_"], func=AF.Tanh, scale=0.5))
                        step(lambda v: nc.vector.tensor_add(
                            v["dbd"], v["ps"][:, :, FSLOT:3 * FSLOT],
                            v["bias"][:, :, FSLOT:3 * FSLOT]))
                        step(lambda v: nc.vector.scalar_tensor_tensor(
                            out=v["d_"], in0=v["tmp"], scalar=1.0, in1=v["d_"],
                            op0=ALU.add, op1=ALU.mult))          # v1 = 2*d1
                        step(lambda v: nc.scalar.activation(
                            out=v["tmp"], in_=v["d_"], func=AF.Tanh, scale=0.25))
                        step(lambda v: nc.vector.scalar_tensor_tensor(
                            out=v["bd_"], in0=v["tmp"], scalar=1.0, in1=v["bd_"],
                            op0=ALU.add, op1=ALU.mult))          # v2 = 2*bd1
                        step(lambda v: nc.scalar.activation(
                            out=v["tmp"], in_=v["bd_"], func=AF.Tanh, scale=0.25))
                        step(lambda v: nc.vector.scalar_tensor_tensor(
                            out=v["d_"], in0=v["tmp"], scalar=1.0, in1=v["d_"],
                            op0=ALU.add, op1=ALU.add))           # w = 2*d2
                        step(lambda v: nc.scalar.activation(
                            out=v["tmp"], in_=v["d_"], func=AF.Tanh, scale=0.25))
                        step(lambda v: nc.vector.scalar_tensor_tensor(
                            out=v["m_"], in0=v["tmp"], scalar=0.5, in1=v["m_"],
                            op0=ALU.mult, op1=ALU.add))          # p1 = probs - 0.5
                        # E = exp(probs + ESHIFT) -> fp8 (range safe for e4m3)
                        step(lambda v: nc.scalar.activation(
                            out=v["e"], in_=v["m_"], func=AF.Exp,
                            bias=eshift_bias))

                    if "chain" in feats and "ocr" in feats:
                        for g in range(NGRP - OLAG, NGRP):
                            emit_ocr(g)

                # ---------------- head ----------------
                if "head" not in feats:
                    finwts = head_sb.tile([128, CB, N], BF16, name="finwts_nb")
                    nc.vector.tensor_copy(out=finwts, in_=finwt_sb)
                else:
                  with contextlib.ExitStack() as hctx:
                      ph_pool = hctx.enter_context(
                          tc.tile_pool(name="ph", bufs=3, space="PSUM"))
                      pht_pool = hctx.enter_context(
                          tc.tile_pool(name="pht", bufs=1, space="PSUM"))
                      hs = head_sb

                      # softmax denominators -> [1, N] -> recip -> [N, 1]
                      sums_v = psum_sums.rearrange("o (j n) -> o n j", n=FS)
                      rsum_row = hs.tile([1, FS], F32, name="rsum_row")
                      nc.vector.tensor_reduce(out=rsum_row, in_=sums_v,
                                              axis=mybir.AxisListType.X, op=ALU.add)
                      nc.vector.reciprocal(out=rsum_row[:, 0:N], in_=rsum_row[:, 0:N])
                      ps_rsT = ph_pool.tile([N, 1], F32, name="ps_rsT", tag="ph")
                      nc.tensor.matmul(ps_rsT, rsum_row[:, 0:N], one1,
                                       start=True, stop=True)
                      rsum_col = hs.tile([N, 1], F32, name="rsum_col")
                      nc.scalar.copy(out=rsum_col, in_=ps_rsT)

                      # normalized ocr straight from the DR psum accumulator
                      ocr_sb = hs.tile([N, C], F32, name="ocr_sb")
                      nc.vector.tensor_scalar(out=ocr_sb, in0=psum_ocrp[0:N, :],
                                              scalar1=rsum_col, scalar2=None,
                                              op0=ALU.mult)

                      # attention over N
                      att_scr = hs.tile([N, C], F32, name="att_scr")
                      att_raw = hs.tile([N, 1], F32, name="att_raw")
                      nc.vector.scalar_tensor_tensor(
                          out=att_scr, in0=ocr_sb, scalar=1.0, in1=maskw_sb,
                          op0=ALU.mult, op1=ALU.mult, accum_out=att_raw)
                      eatt = hs.tile([N, 1], F32, name="eatt")
                      nc.scalar.activation(out=eatt, in_=att_raw, func=AF.Exp)
                      ps_s = ph_pool.tile([1, 1], F32, name="ps_s", tag="ph")
                      nc.tensor.matmul(ps_s, eatt, ones19, start=True, stop=True)
                      srecip = hs.tile([1, 1], F32, name="srecip")
                      nc.vector.reciprocal(out=srecip, in_=ps_s)

                      # ctx = (ocr^T @ eatt) / sum  -> [1, C] -> transpose [128, 4]
                      ps_ctx = ph_pool.tile([1, C], F32, name="ps_ctx", tag="ph")
                      nc.tensor.matmul(ps_ctx, eatt, ocr_sb, start=True, stop=True)
                      ctx_sb = hs.tile([1, C], F32, name="ctx_sb")
                      nc.vector.tensor_scalar(out=ctx_sb, in0=ps_ctx, scalar1=srecip,
                                              scalar2=None, op0=ALU.mult)
                      ps_ctxT = ph_pool.tile([128, CB], F32, name="ps_ctxT", tag="ph")
                      for cb in range(CB):
                          nc.tensor.matmul(ps_ctxT[:, cb:cb + 1],
                                           ctx_sb[0:1, cb * 128:(cb + 1) * 128], one1,
                                           start=True, stop=True)
                      ctxT_sb = hs.tile([128, CB], BF16, name="ctxT_sb")
                      nc.scalar.copy(out=ctxT_sb, in_=ps_ctxT)

                      # t = cm1 @ ctx + b
                      ps_t = pht_pool.tile([128, CB], F32, name="ps_t", tag="pht")
                      for mj in range(CB):
                          for cb in range(CB):
                              nc.tensor.matmul(
                                  ps_t[:, mj:mj + 1],
                                  cm1t_sb[:, cb, mj * 128:(mj + 1) * 128],
                                  ctxT_sb[:, cb:cb + 1],
                                  start=(cb == 0), stop=(cb == CB - 1))
                      t8 = hs.tile([128, 2 * CB], F32, name="t8")
                      t_sb = t8[:, 0:CB]
                      nc.vector.tensor_add(t_sb, ps_t, cm1b_sb)

                      # layernorm stats over all 512
                      sq_sb = t8[:, CB:2 * CB]
                      nc.scalar.activation(out=sq_sb, in_=t_sb, func=AF.Square)
                      ps_st = ph_pool.tile([1, 2 * CB], F32, name="ps_st", tag="ph")
                      nc.tensor.matmul(ps_st, ones_col, t8, start=True, stop=True)
                      s1 = hs.tile([1, 1], F32, name="s1")
                      nc.vector.tensor_reduce(out=s1, in_=ps_st[:, 0:CB],
                                              axis=mybir.AxisListType.X, op=ALU.add)
                      s2 = hs.tile([1, 1], F32, name="s2")
                      nc.vector.tensor_reduce(out=s2, in_=ps_st[:, CB:2 * CB],
                                              axis=mybir.AxisListType.X, op=ALU.add)
                      mr = hs.tile([1, 2], F32, name="mr")
                      mu = mr[:, 0:1]
                      nc.vector.tensor_scalar(out=mu, in0=s1, scalar1=1.0 / C,
                                              scalar2=None, op0=ALU.mult)
                      ms2e = hs.tile([1, 1], F32, name="ms2e")
                      nc.vector.tensor_scalar(out=ms2e, in0=s2, scalar1=1.0 / C,
                                              scalar2=LN_EPS, op0=ALU.mult,
                                              op1=ALU.add)
                      mu2 = hs.tile([1, 1], F32, name="mu2")
                      nc.vector.tensor_mul(mu2, mu, mu)
                      vpe = hs.tile([1, 1], F32, name="vpe")
                      nc.vector.tensor_sub(vpe, ms2e, mu2)

                      # rsq = rsqrt(vpe) via DVE bit-trick + 3 Newton steps
                      rsq = mr[:, 1:2]
                      sh_u = hs.tile([1, 1], U32, name="sh_u")
                      nc.vector.tensor_tensor(out=sh_u, in0=vpe.bitcast(U32),
                                              in1=one_u,
                                              op=ALU.logical_shift_right)
                      nc.vector.tensor_tensor(out=rsq.bitcast(U32), in0=magic_u,
                                              in1=sh_u, op=ALU.subtract)
                      nt = hs.tile([1, 1], F32, name="nt")
                      for _ in range(3):
                          nc.vector.tensor_mul(nt, vpe, rsq)
                          nc.vector.tensor_mul(nt, nt, rsq)
                          nc.vector.scalar_tensor_tensor(
                              out=nt, in0=nt, scalar=-0.5, in1=c15,
                              op0=ALU.mult, op1=ALU.add)   # 1.5 - 0.5*v*y^2
                          nc.vector.tensor_mul(rsq, rsq, nt)

                      # broadcast [mu, rsq] to 128 partitions in one matmul
                      ps_mr = ph_pool.tile([128, 2], F32, name="ps_mr", tag="ph")
                      nc.tensor.matmul(ps_mr, ones_row, mr, start=True, stop=True)
                      mr_bc = hs.tile([128, 2], F32, name="mr_bc")
                      nc.scalar.copy(out=mr_bc, in_=ps_mr)

                      z_sb = hs.tile([128, CB], F32, name="z_sb")
                      nc.vector.tensor_scalar(out=z_sb, in0=t_sb,
                                              scalar1=mr_bc[:, 0:1],
                                              scalar2=mr_bc[:, 1:2],
                                              op0=ALU.subtract,
                                              op1=ALU.mult)
                      nc.vector.tensor_mul(z_sb, z_sb, lng_sb)
                      nc.vector.tensor_add(z_sb, z_sb, lnb_sb)
                      z_bf = hs.tile([128, CB], BF16, name="z_bf")
                      nc.vector.tensor_scalar_max(z_bf, z_sb, 0.0)   # relu

                      # t2 = cm2 @ relu + b ; gate = sigmoid(t2) via exp
                      ps_t2 = pht_pool.tile([128, CB], F32, name="ps_t2", tag="pht")
                      for cj in range(CB):
                          for mb in range(CB):
                              nc.tensor.matmul(
                                  ps_t2[:, cj:cj + 1],
                                  cm2t_sb[:, mb, cj * 128:(cj + 1) * 128],
                                  z_bf[:, mb:mb + 1],
                                  start=(mb == 0), stop=(mb == CB - 1))
                      s2t = hs.tile([128, CB], F32, name="s2t")
                      nc.vector.tensor_add(s2t, ps_t2, cm2b_sb)
                      ex = hs.tile([128, CB], F32, name="ex")
                      nc.scalar.activation(out=ex, in_=s2t, func=AF.Exp, scale=-1.0)
                      nc.vector.tensor_scalar_add(ex, ex, 1.0)
                      gate = hs.tile([128, CB], F32, name="gate")
                      nc.vector.reciprocal(out=gate, in_=ex)
                      scale_vec = hs.tile([128, CB], F32, name="scale_vec")
                      nc.vector.tensor_scalar_add(scale_vec, gate, 1.0)

                      finwts = hs.tile([128, CB, N], BF16, name="finwts")
                      for cb in range(CB):
                          nc.vector.tensor_scalar(
                              out=finwts[:, cb, :], in0=finwt_sb[:, cb, :],
                              scalar1=scale_vec[:, cb:cb + 1], scalar2=None,
                              op0=ALU.mult)

                # ---------------- final conv ----------------
                if "fin" not in feats:
                    return
                with contextlib.ExitStack() as fctx:
                    psF_pool = fctx.enter_context(
                        tc.tile_pool(name="psF", bufs=3, space="PSUM"))
                    if "finct" in feats:
                        # 4-way col-tiled: 4 pc's concurrently in col-groups
                        for blk in range(NPC // 4):
                            outt = out_ring.tile([N, 4 * PCW], BF16, name="outt")
                            psF4 = psF_pool.tile([128, PCW], F32, name="psF4")
                            for cb in range(CB):
                                for q in range(4):
                                    pc = blk * 4 + q
                                    nc.tensor.matmul(
                                        psF4[32 * q:32 * q + N, :],
                                        finwts[:, cb, :],
                                        x_str[blk][:, cb, q * PCW:(q + 1) * PCW],
                                        start=(cb == 0), stop=(cb == CB - 1),
                                        tile_position=(0, 32 * q),
                                        skip_group_check=True)
                            for q in range(4):
                                dst = outt[:, q * PCW:(q + 1) * PCW]
                                src = psF4[32 * q:32 * q + N, :]
                                if q % 2 == 0:
                                    nc.scalar.activation(
                                        out=dst, in_=src, func=AF.Identity,
                                        bias=finb_bc[32 * q:32 * q + N, :])
                                else:
                                    nc.vector.tensor_scalar_add(
                                        dst, src, finb_bc[32 * q:32 * q + N, :])
                            c0 = blk * 4 * PCW
                            nc.gpsimd.dma_start(
                                out=out_d.ap()[:, c0:c0 + 4 * PCW], in_=outt)
                    else:
                        for pc in range(NPC):
                            if pc % DMAC == 0:
                                outt = out_ring.tile([N, DMAC * PCW], BF16,
                                                     name="outt")
                            psF = psF_pool.tile([N, PCW], F32, name="psF")
                            for cb in range(CB):
                                nc.tensor.matmul(
                                    psF, finwts[:, cb, :],
                                    x_str[pc // 4][:, cb, (pc % 4) * PCW:
                                                   (pc % 4 + 1) * PCW],
                                    start=(cb == 0), stop=(cb == CB - 1))
                            dst = outt[:, (pc % DMAC) * PCW:(pc % DMAC + 1) * PCW]
                            if pc % 2 == 0:
                                nc.scalar.activation(out=dst, in_=psF,
                                                     func=AF.Identity,
                                                     bias=finb_sb)
                            else:
                                nc.vector.tensor_scalar_add(dst, psF, finb_sb)
                            if pc % DMAC == DMAC - 1:
                                c0 = (pc - DMAC + 1) * PCW
                                nc.gpsimd.dma_start(
                                    out=out_d.ap()[:, c0:c0 + DMAC * PCW],
                                    in_=outt)

            if repeat == 1:
                main_body()
            elif repeat == 2:
                main_body()
                main_body()
            else:
                with tc.For_i(0, repeat // 2, 1, staggered_reset=True):
                    main_body()
                    main_body()
                if repeat % 2:
                    main_body()

    nc.compile()
    return nc


_cached = {}


def _get_module(repeat=1, feats=("ident", "chain", "ocr", "head", "fin",
                                 "evacb", "xload")):
    key = (repeat, frozenset(feats))
    if key not in _cached:
        _cached[key] = build_module(repeat, feats)
    return _cached[key]


def prep_weights(inputs):
    f = np.float32
    map_w = np.asarray(inputs["map_w"], f)
    dist_w = np.asarray(inputs["dist_w"], f)
    bnd_w = np.asarray(inputs["bnd_w"], f)
    wt60 = np.zeros((C, 3 * FSLOT), f)
    wt60[:, 0:N] = map_w.T
    wt60[:, FSLOT:FSLOT + N] = dist_w.T
    wt60[:, 2 * FSLOT:2 * FSLOT + N] = bnd_w.T
    bias60 = np.zeros((3 * FSLOT,), f)
    bias60[0:N] = np.asarray(inputs["map_b"], f)
    bias60[FSLOT:FSLOT + N] = np.asarray(inputs["dist_b"], f)
    bias60[2 * FSLOT:2 * FSLOT + N] = np.asarray(inputs["bnd_b"], f)
    shared = {
        "wt60": wt60,
        "bias60": bias60,
        "maskw": np.asarray(inputs["mask_w"], f),
        "cm1T": np.ascontiguousarray(np.asarray(inputs["cm1_w"], f).T),
        "cm1b": np.asarray(inputs["cm1_b"], f),
        "lng": np.asarray(inputs["ln_g"], f),
        "lnb": np.asarray(inputs["ln_b"], f),
        "cm2T": np.ascontiguousarray(np.asarray(inputs["cm2_w"], f).T),
        "cm2b": np.asarray(inputs["cm2_b"], f),
        "finWT": np.ascontiguousarray(np.asarray(inputs["fin_w"], f).T),
        "finb": np.pad(np.asarray(inputs["fin_b"], f), (0, 32 - N)),
    }
    return shared


def make_in_maps(inputs):
    """Per-core input dicts: weights (shared) + per-sample x (bf16) and
    xT (fp8, SBUF layout [128, NCH*C])."""
    import ml_dtypes
    shared = prep_weights(inputs)
    x = np.asarray(inputs["x"], np.float32)
    in_maps = []
    for b in range(B):
        m = dict(shared)
        xb = x[b].reshape(C, HW)
        m["x"] = np.ascontiguousarray(xb).astype(ml_dtypes.bfloat16)
        xt = np.ascontiguousarray(
            xb.reshape(C, NCH, 128).transpose(2, 1, 0))  # [128, NCH, C]
        m["xt8"] = xt.astype(ml_dtypes.float8_e4m3).reshape(128, NCH * C)
        in_maps.append(m)
    return in_maps


def kernel(**inputs):
    nc = _get_module(1)
    in_maps = make_in_maps(inputs)
    res = bass_utils.run_bass_kernel_spmd(nc, in_maps, core_ids=list(range(B)))
    out = np.stack([np.asarray(res.results[b]["out"]).astype(np.float32)
                    .reshape(N, H, W) for b in range(B)])
    return out.astype(np.float32)

